# revision 18
# baseline (speedup 1.0000x reference)
"""Multi-head attention kernel for Trainium2 (8 NeuronCores, Bass/Tile).

Problem: B=2, S=2048, D=1024, H=16 heads (HD=64), causal mask, fp32.
Reference quirk: V is projected from the ALREADY-projected keys:
    k = keys @ Wk + bk ; v = k @ Wv + bv  =>  v = keys @ (Wk@Wv) + (bk@Wv + bv)

Sharding: core c handles batch b = c//4 and head-group g = c%4 (4 heads,
head-feature columns [256g, 256g+256)).  Each core:
  - projects q/k/v for its heads from its batch (contraction over full D),
  - computes full-sequence attention for its 4 heads,
  - produces a partial output  attn_g @ Wo[rows of g]  (row-parallel Wo).
Host sums the 4 partials per batch and adds bo.

Device layouts (per core):
  xqT/xkT   [D, S]  fp32   (host-transposed activations)
  qT/kT     [128, 2*S] bf16  (head-feat on partitions; hf-block hb -> cols hb*S+tok)
  v_aug     [128, NKC*512] bf16 (per k-chunk of 128 toks: per head [v_h(64)|ones(64)])
  S^T tile  psum [128 ktok, 512 qtok] = kT_h.T @ qT_h   (contraction over HD=64)
  P^T tile  bf16 [128, 512] = exp(S^T/8) with causal zeroing/masking
  PV        psum [128, 512] = v_aug_h.T @ P^T  accum over k-chunks:
              rows 0:64 = unnormalized attn^T, rows 64:128 = replicated row-sums
  attnT     [128, 2*S] bf16 = normalized attn^T  (DVE: rows0:64 * recip(rows64:128))
  out chunk psum [128 tok, 512 of] = attnT.T @ Wo  accum over 2 hf-blocks
"""
import sys
sys.path.insert(0, "/opt/trn_rl_repo")

import numpy as np
import ml_dtypes

import concourse.bacc as bacc
import concourse.mybir as mybir
import concourse.tile as tile
from concourse.bass_utils import run_bass_kernel_spmd

F32 = mybir.dt.float32
F32R = mybir.dt.float32r
BF16 = mybir.dt.bfloat16
AF = mybir.ActivationFunctionType

B, S, D, H, HD = 2, 2048, 1024, 16, 64
NCORES = 8
HPC = 4            # heads per core
HF = HPC * HD      # 256 head-features per core
NKC = S // 128     # 16 k-chunks of 128 tokens
NQB = S // 512     # 4 q-blocks of 512 tokens
NDC = D // 128     # 8 contraction chunks for projections
SCALE = 1.0 / np.sqrt(HD)


def _classify_mask(mask):
    """Per (qblock 512, kchunk 128) x (qsub 128) classification of mask^T.

    Returns (plan, mask_tiles):
      plan[qb][kc] = None (fully masked -> skip) or (subs, c0, c1) where
        subs[j] in {('Z',), ('F',), ('M', idx)} and [c0, c1) is the exp span.
      mask_tiles: list of distinct [128,128] 0/1 int tiles (transposed: [kt, qt]).
    """
    maskT = np.ascontiguousarray(mask.T)
    tiles = {}
    tiles_list = []
    plan = []
    for qb in range(NQB):
        row = []
        for kc in range(NKC):
            subT = maskT[kc * 128:(kc + 1) * 128, qb * 512:(qb + 1) * 512]
            subs = []
            for j in range(4):
                blk = subT[:, j * 128:(j + 1) * 128]
                if not blk.any():
                    subs.append(('Z',))
                elif blk.all():
                    subs.append(('F',))
                else:
                    key = blk.tobytes()
                    if key not in tiles:
                        tiles[key] = len(tiles_list)
                        tiles_list.append(blk)
                    subs.append(('M', tiles[key]))
            if all(s[0] == 'Z' for s in subs):
                row.append(None)
            else:
                nz = [j for j, s in enumerate(subs) if s[0] != 'Z']
                row.append((subs, nz[0] * 128, (nz[-1] + 1) * 128))
        plan.append(row)
    return plan, tiles_list


def _build_nc(plan, nmt, has_vbias, exp_group=2, cast_engine="act",
              xbufs=10, ptbufs=4, ablate=()):
    ablate = set(ablate)
    nc = bacc.Bacc("TRN2", target_bir_lowering=False, debug=False)

    xqT = nc.dram_tensor("xqT", [D, S], F32R, kind="ExternalInput").ap()
    xkT = nc.dram_tensor("xkT", [D, S], F32R, kind="ExternalInput").ap()
    wq_d = nc.dram_tensor("wq", [D, HF], F32R, kind="ExternalInput").ap()
    wk_d = nc.dram_tensor("wk", [D, HF], F32R, kind="ExternalInput").ap()
    wkv_d = nc.dram_tensor("wkv", [D, HF], F32R, kind="ExternalInput").ap()
    wo_d = nc.dram_tensor("wo", [HF, D], BF16, kind="ExternalInput").ap()
    bq_d = nc.dram_tensor("bq", [128, 2], F32, kind="ExternalInput").ap()
    bk_d = nc.dram_tensor("bk", [128, 2], F32, kind="ExternalInput").ap()
    bkv_d = nc.dram_tensor("bkv", [1, HF], F32R, kind="ExternalInput").ap()
    mt_d = nc.dram_tensor("mtiles", [max(nmt, 1), 128, 128], BF16,
                          kind="ExternalInput").ap()
    out_d = nc.dram_tensor("out", [S, D], F32, kind="ExternalOutput").ap()

    with tile.TileContext(nc) as tc:
        with tc.tile_pool(name="wpool", bufs=1) as wpool, \
             tc.tile_pool(name="big", bufs=1) as big, \
             tc.tile_pool(name="xpool", bufs=xbufs) as xpool, \
             tc.tile_pool(name="ptpool", bufs=ptbufs) as ptpool, \
             tc.tile_pool(name="npool", bufs=2) as npool, \
             tc.tile_pool(name="opool", bufs=3) as opool, \
             tc.tile_pool(name="pspool", bufs=4, space="PSUM") as pspool, \
             tc.tile_pool(name="pspool2", bufs=2, space="PSUM") as pspool2:

            def cast_bias(dst, src, bias_ap):
                if cast_engine == "act":
                    nc.scalar.activation(dst, src, AF.Identity,
                                         bias=bias_ap, scale=1.0)
                else:
                    nc.vector.tensor_scalar_add(dst, src, bias_ap)

            # ---------------- weights / constants ----------------
            # Loaded just-in-time (emission order = DMA queue order): wq/bq
            # before q-proj, wk/wkv before kv-proj, wo/masks before first use.
            wq_sb = wpool.tile([128, NDC * HF], F32R, tag="wq")
            wk_sb = wpool.tile([128, NDC * HF], F32R, tag="wk")
            wkv_sb = wpool.tile([128, NDC * HF], F32R, tag="wkv")
            wo_sb = wpool.tile([128, 2 * D], BF16, tag="wo")
            bq_sb = wpool.tile([128, 2], F32, tag="bq")
            bk_sb = wpool.tile([128, 2], F32, tag="bk")
            mt_sb = None
            if nmt > 0:
                mt_sb = wpool.tile([128, nmt * 128], BF16, tag="mt",
                                   name="mt_sb")
            if has_vbias:
                ones_sb = wpool.tile([1, 128], F32R, tag="ones")
                bkv_sb = wpool.tile([1, HF], F32R, tag="bkv")

            def emit_load_q_weights():
                nc.sync.dma_start(bq_sb[:], bq_d)
                for kc in range(NDC):
                    nc.sync.dma_start(wq_sb[:, kc * HF:(kc + 1) * HF],
                                      wq_d[kc * 128:(kc + 1) * 128, :])

            def emit_load_kv_weights(kc):
                if kc == 0:
                    nc.sync.dma_start(bk_sb[:], bk_d)
                nc.sync.dma_start(wk_sb[:, kc * HF:(kc + 1) * HF],
                                  wk_d[kc * 128:(kc + 1) * 128, :])
                nc.sync.dma_start(wkv_sb[:, kc * HF:(kc + 1) * HF],
                                  wkv_d[kc * 128:(kc + 1) * 128, :])
                if has_vbias and kc == 0:
                    nc.gpsimd.memset(ones_sb[:], 1.0)
                    nc.sync.dma_start(bkv_sb[:], bkv_d)

            def emit_load_masks():
                for i in range(nmt):
                    nc.sync.dma_start(mt_sb[:, i * 128:(i + 1) * 128], mt_d[i])

            def emit_load_wo():
                for hb in range(2):
                    nc.sync.dma_start(wo_sb[:, hb * D:(hb + 1) * D],
                                      wo_d[hb * 128:(hb + 1) * 128, :])

            # ---------------- persistent activations ----------------
            qT_sb = big.tile([128, 2 * S], BF16, tag="qT")
            kT_sb = big.tile([128, 2 * S], BF16, tag="kT")
            vaug_sb = big.tile([128, NKC * 512], BF16, tag="vaug")
            attnT_sb = big.tile([128, 2 * S], BF16, tag="attnT")

            # ones blocks of v_aug: per kchunk, per head: cols [.. +64 .. +128)
            for kc in range(NKC):
                for h in range(HPC):
                    nc.gpsimd.memset(
                        vaug_sb[:, kc * 512 + h * 128 + 64: kc * 512 + h * 128 + 128],
                        1.0)

            # ---------------- projection emitters ----------------
            # q projection for one 512-token chunk: two per-hb psum singles.
            def emit_qproj(tci, weave_dma=None):
                psq = [pspool.tile([128, 512], F32, tag="ps",
                                   name=f"psq{tci}_{hb}") for hb in range(2)]
                for kc in range(NDC):
                    xq_t = xpool.tile([128, 512], F32R, tag="x",
                                      name=f"xq{tci}_{kc}")
                    nc.sync.dma_start(
                        xq_t[:], xqT[kc * 128:(kc + 1) * 128,
                                     tci * 512:(tci + 1) * 512])
                    if weave_dma is not None:
                        weave_dma(kc)
                    for hb in range(2):
                        nc.tensor.matmul(
                            psq[hb][:],
                            wq_sb[:, kc * HF + hb * 128: kc * HF + (hb + 1) * 128],
                            xq_t[:],
                            start=(kc == 0), stop=(kc == NDC - 1))
                for hb in range(2):
                    cast_bias(
                        qT_sb[:, hb * S + tci * 512: hb * S + (tci + 1) * 512],
                        psq[hb][:], bq_sb[:, hb:hb + 1])

            # k+v projections for one 512-token chunk; xk tiles stay live
            # across the kc loop so v accumulates one psv at a time.
            def emit_kvproj(tci):
                psk = [pspool.tile([128, 512], F32, tag="ps",
                                   name=f"psk{tci}_{hb}") for hb in range(2)]
                xks = []
                for kc in range(NDC):
                    xk_t = xpool.tile([128, 512], F32R, tag="x",
                                      name=f"xk{tci}_{kc}")
                    nc.sync.dma_start(
                        xk_t[:], xkT[kc * 128:(kc + 1) * 128,
                                     tci * 512:(tci + 1) * 512])
                    xks.append(xk_t)
                    for hb in range(2):
                        nc.tensor.matmul(
                            psk[hb][:],
                            wk_sb[:, kc * HF + hb * 128: kc * HF + (hb + 1) * 128],
                            xk_t[:],
                            start=(kc == 0), stop=(kc == NDC - 1))
                for hb in range(2):
                    cast_bias(
                        kT_sb[:, hb * S + tci * 512: hb * S + (tci + 1) * 512],
                        psk[hb][:], bk_sb[:, hb:hb + 1])
                for ts in range(4):
                    psv = pspool.tile([128, HF], F32, tag="ps",
                                      name=f"psv{tci}_{ts}")
                    for kc in range(NDC):
                        nc.tensor.matmul(
                            psv[:],
                            xks[kc][:, ts * 128:(ts + 1) * 128],
                            wkv_sb[:, kc * HF:(kc + 1) * HF],
                            start=(kc == 0),
                            stop=(kc == NDC - 1 and not has_vbias))
                    if has_vbias:
                        nc.tensor.matmul(psv[:], ones_sb[:], bkv_sb[:],
                                         start=False, stop=True)
                    kci = tci * 4 + ts
                    for h in range(HPC):
                        nc.vector.tensor_copy(
                            vaug_sb[:, kci * 512 + h * 128: kci * 512 + h * 128 + 64],
                            psv[:, h * 64:(h + 1) * 64])

            # output projection for one 128-token chunk of a finished q-block
            def emit_oproj(tok0):
                for of in range(2):
                    ops = pspool.tile([128, 512], F32, tag="ps")
                    for hb2 in range(2):
                        nc.tensor.matmul(
                            ops[:],
                            attnT_sb[:, hb2 * S + tok0: hb2 * S + tok0 + 128],
                            wo_sb[:, hb2 * D + of * 512: hb2 * D + (of + 1) * 512],
                            start=(hb2 == 0), stop=(hb2 == 1))
                    obuf = opool.tile([128, 512], F32, tag="obuf")
                    nc.vector.tensor_copy(obuf[:], ops[:])
                    nc.sync.dma_start(
                        out_d[tok0:tok0 + 128, of * 512:(of + 1) * 512],
                        obuf[:])

            # ---------------- attention ----------------
            # Pair k-chunks so one wide ACT exp covers two 512-col S^T tiles
            # (amortizes the ~352-cycle ACT instruction setup).
            def emit_attention(qb, extras):
                q0 = qb * 512
                kcs = [kc for kc in range(NKC) if plan[qb][kc] is not None]
                groups = [kcs[i:i + exp_group]
                          for i in range(0, len(kcs), exp_group)]
                for h in range(HPC):
                    hb, hr = h // 2, (h % 2) * 64
                    pv_ps = pspool.tile([128, 512], F32, tag="ps",
                                        name=f"pv{qb}_{h}")
                    for grp in groups:
                        g = len(grp)
                        pool_g = pspool2 if g > 1 else pspool
                        st_ps = pool_g.tile([128, 512 * g], F32,
                                            tag="ps2" if g > 1 else "ps")
                        pt = ptpool.tile([128, 512 * g], BF16, tag="pt")
                        for i, kc in enumerate(grp):
                            nc.tensor.matmul(
                                st_ps[:, i * 512:(i + 1) * 512],
                                kT_sb[hr:hr + 64,
                                      hb * S + kc * 128: hb * S + (kc + 1) * 128],
                                qT_sb[hr:hr + 64, hb * S + q0: hb * S + q0 + 512],
                                start=True, stop=True)
                        e0 = plan[qb][grp[0]][1]
                        e1 = (g - 1) * 512 + plan[qb][grp[-1]][2]
                        nc.scalar.activation(pt[:, e0:e1], st_ps[:, e0:e1],
                                             AF.Exp, bias=0.0,
                                             scale=float(SCALE))
                        for i, kc in enumerate(grp):
                            subs, c0, c1 = plan[qb][kc]
                            base = i * 512
                            # PV streams only [c0, c1); cols outside stay
                            # unread (masked-out contributions are skipped,
                            # not zeroed).  Interior all-zero subs still need
                            # a memset.  The overall-first k-chunk must cover
                            # the full 512 so every psum column is started.
                            if kc == kcs[0] and (c0 != 0 or c1 != 512):
                                if c0 > 0:
                                    nc.vector.memset(pt[:, base:base + c0], 0.0)
                                if c1 < 512:
                                    nc.vector.memset(pt[:, base + c1:base + 512], 0.0)
                                c0, c1 = 0, 512
                            for j, sub in enumerate(subs):
                                lo, hi = base + j * 128, base + (j + 1) * 128
                                if sub[0] == 'Z' and j * 128 >= c0 and (j + 1) * 128 <= c1:
                                    nc.vector.memset(pt[:, lo:hi], 0.0)
                                elif sub[0] == 'M':
                                    idx = sub[1]
                                    nc.vector.tensor_mul(
                                        pt[:, lo:hi], pt[:, lo:hi],
                                        mt_sb[:, idx * 128:(idx + 1) * 128])
                            nc.tensor.matmul(
                                pv_ps[:, c0:c1],
                                vaug_sb[:, kc * 512 + h * 128: kc * 512 + (h + 1) * 128],
                                pt[:, base + c0:base + c1],
                                start=(kc == kcs[0]), stop=(kc == kcs[-1]))
                    recip = npool.tile([64, 512], F32, tag="recip")
                    nc.vector.reciprocal(recip[:], pv_ps[64:128, :])
                    nc.vector.tensor_mul(
                        attnT_sb[hr:hr + 64, hb * S + q0: hb * S + q0 + 512],
                        pv_ps[0:64, :], recip[:])
                    if extras:
                        extras.pop(0)()
                while extras:
                    extras.pop(0)()

            # ---------------- emission order (weaving) ----------------
            # Early projections, then per q-block attention with later
            # projections / previous q-block output projections woven between
            # heads so PE has work while ACT chews through the exps.
            emit_load_q_weights()
            emit_qproj(0, weave_dma=emit_load_kv_weights)
            emit_kvproj(0)
            emit_load_masks()
            emit_attention(0, [lambda: emit_load_wo(),
                               lambda: emit_qproj(1), lambda: emit_kvproj(1),
                               lambda: emit_qproj(2), lambda: emit_kvproj(2)])
            emit_attention(1, [lambda: emit_qproj(3), lambda: emit_kvproj(3),
                               lambda: emit_oproj(0 * 512 + 0 * 128),
                               lambda: emit_oproj(0 * 512 + 1 * 128)])
            emit_attention(2, [lambda: emit_oproj(0 * 512 + 2 * 128),
                               lambda: emit_oproj(0 * 512 + 3 * 128),
                               lambda: emit_oproj(1 * 512 + 0 * 128),
                               lambda: emit_oproj(1 * 512 + 1 * 128)])
            emit_attention(3, [lambda: emit_oproj(1 * 512 + 2 * 128),
                               lambda: emit_oproj(1 * 512 + 3 * 128),
                               lambda: emit_oproj(2 * 512 + 0 * 128),
                               lambda: emit_oproj(2 * 512 + 1 * 128),
                               lambda: emit_oproj(2 * 512 + 2 * 128),
                               lambda: emit_oproj(2 * 512 + 3 * 128)])
            for t in range(4):
                emit_oproj(3 * 512 + t * 128)
    nc.compile()
    return nc


_CACHE = {}

# tunables (kept as module globals so experiments can override)
VARIANT = {"exp_group": 2, "cast_engine": "act", "xbufs": 10, "ptbufs": 4}


def _get_nc(plan, nmt, has_vbias):
    key = (repr(plan), nmt, has_vbias, repr(sorted(VARIANT.items())))
    if key not in _CACHE:
        _CACHE[key] = _build_nc(plan, nmt, has_vbias, **VARIANT)
    return _CACHE[key]


def shard_inputs(queries, keys, mask, Wq, bq, Wk, bk, Wv, bv, Wo, bo):
    """Host-side prep: returns (in_maps, plan, nmt, has_vbias)."""
    Wkv = (Wk.astype(np.float64) @ Wv.astype(np.float64)).astype(np.float32)
    bkv = (bk.astype(np.float64) @ Wv.astype(np.float64)
           + bv.astype(np.float64)).astype(np.float32)
    has_vbias = bool(np.any(bkv != 0.0))

    plan, tiles_list = _classify_mask(np.asarray(mask))
    nmt = len(tiles_list)
    assert nmt <= 64, f"too many distinct mask tiles ({nmt})"
    if nmt > 0:
        mtiles = np.stack(tiles_list).astype(ml_dtypes.bfloat16)
    else:
        mtiles = np.zeros((1, 128, 128), dtype=ml_dtypes.bfloat16)

    in_maps = []
    for c in range(NCORES):
        b, g = c // 4, c % 4
        cols = slice(HF * g, HF * (g + 1))
        in_maps.append({
            "xqT": np.ascontiguousarray(queries[b].T),
            "xkT": np.ascontiguousarray(keys[b].T),
            "wq": np.ascontiguousarray(Wq[:, cols]),
            "wk": np.ascontiguousarray(Wk[:, cols]),
            "wkv": np.ascontiguousarray(Wkv[:, cols]),
            "wo": np.ascontiguousarray(Wo[cols, :]).astype(ml_dtypes.bfloat16),
            "bq": np.ascontiguousarray(bq[cols].reshape(2, 128).T),
            "bk": np.ascontiguousarray(bk[cols].reshape(2, 128).T),
            "bkv": bkv[cols].reshape(1, HF).copy(),
            "mtiles": mtiles,
        })
    return in_maps, plan, nmt, has_vbias


def combine_outputs(results, bo):
    out = np.empty((B, S, D), dtype=np.float32)
    for b in range(B):
        acc = results[4 * b]["out"].astype(np.float32).copy()
        for g in range(1, 4):
            acc += results[4 * b + g]["out"]
        out[b] = acc + bo[None, :]
    return out


def kernel(queries, keys, values, mask, Wq, bq, Wk, bk, Wv, bv, Wo, bo,
           _trace=False, _result_holder=None):
    queries = np.asarray(queries, dtype=np.float32)
    keys = np.asarray(keys, dtype=np.float32)
    mask = np.asarray(mask)
    in_maps, plan, nmt, has_vbias = shard_inputs(
        queries, keys, mask,
        np.asarray(Wq, np.float32), np.asarray(bq, np.float32),
        np.asarray(Wk, np.float32), np.asarray(bk, np.float32),
        np.asarray(Wv, np.float32), np.asarray(bv, np.float32),
        np.asarray(Wo, np.float32), np.asarray(bo, np.float32))
    nc = _get_nc(plan, nmt, has_vbias)
    res = run_bass_kernel_spmd(nc, in_maps, core_ids=list(range(NCORES)),
                               trace=_trace)
    if _result_holder is not None:
        _result_holder.append(res)
    return combine_outputs(res.results, np.asarray(bo, np.float32))


# revision 19
# speedup vs baseline: 586.1890x; 586.1890x over previous
"""Multi-head attention kernel for Trainium2 (8 NeuronCores, Bass/Tile).

Problem: B=2, S=2048, D=1024, H=16 heads (HD=64), causal mask, fp32.
Reference quirk: V is projected from the ALREADY-projected keys:
    k = keys @ Wk + bk ; v = k @ Wv + bv  =>  v = keys @ (Wk@Wv) + (bk@Wv + bv)

Sharding: core c handles batch b = c//4 and head-group g = c%4 (4 heads,
head-feature columns [256g, 256g+256)).  Each core:
  - projects q/k/v for its heads from its batch (contraction over full D),
  - computes full-sequence attention for its 4 heads,
  - produces a partial output  attn_g @ Wo[rows of g]  (row-parallel Wo).
Host sums the 4 partials per batch and adds bo.

Device layouts (per core):
  xqT/xkT   [D, S]  fp32   (host-transposed activations)
  qT/kT     [128, 2*S] bf16  (head-feat on partitions; hf-block hb -> cols hb*S+tok)
  v_aug     [128, NKC*512] bf16 (per k-chunk of 128 toks: per head [v_h(64)|ones(64)])
  S^T tile  psum [128 ktok, 512 qtok] = kT_h.T @ qT_h   (contraction over HD=64)
  P^T tile  bf16 [128, 512] = exp(S^T/8) with causal zeroing/masking
  PV        psum [128, 512] = v_aug_h.T @ P^T  accum over k-chunks:
              rows 0:64 = unnormalized attn^T, rows 64:128 = replicated row-sums
  attnT     [128, 2*S] bf16 = normalized attn^T  (DVE: rows0:64 * recip(rows64:128))
  out chunk psum [128 tok, 512 of] = attnT.T @ Wo  accum over 2 hf-blocks
"""
import sys
sys.path.insert(0, "/opt/trn_rl_repo")

import numpy as np
import ml_dtypes

import concourse.bacc as bacc
import concourse.mybir as mybir
import concourse.tile as tile
from concourse.bass_utils import run_bass_kernel_spmd

F32 = mybir.dt.float32
F32R = mybir.dt.float32r
BF16 = mybir.dt.bfloat16
AF = mybir.ActivationFunctionType

B, S, D, H, HD = 2, 2048, 1024, 16, 64
NCORES = 8
HPC = 4            # heads per core
HF = HPC * HD      # 256 head-features per core
NKC = S // 128     # 16 k-chunks of 128 tokens
NQB = S // 512     # 4 q-blocks of 512 tokens
NDC = D // 128     # 8 contraction chunks for projections
SCALE = 1.0 / np.sqrt(HD)


def _classify_mask(mask):
    """Per (qblock 512, kchunk 128) x (qsub 128) classification of mask^T.

    Returns (plan, mask_tiles):
      plan[qb][kc] = None (fully masked -> skip) or (subs, c0, c1) where
        subs[j] in {('Z',), ('F',), ('M', idx)} and [c0, c1) is the exp span.
      mask_tiles: list of distinct [128,128] 0/1 int tiles (transposed: [kt, qt]).
    """
    maskT = np.ascontiguousarray(mask.T)
    tiles = {}
    tiles_list = []
    plan = []
    for qb in range(NQB):
        row = []
        for kc in range(NKC):
            subT = maskT[kc * 128:(kc + 1) * 128, qb * 512:(qb + 1) * 512]
            subs = []
            for j in range(4):
                blk = subT[:, j * 128:(j + 1) * 128]
                if not blk.any():
                    subs.append(('Z',))
                elif blk.all():
                    subs.append(('F',))
                else:
                    key = blk.tobytes()
                    if key not in tiles:
                        tiles[key] = len(tiles_list)
                        tiles_list.append(blk)
                    subs.append(('M', tiles[key]))
            if all(s[0] == 'Z' for s in subs):
                row.append(None)
            else:
                nz = [j for j, s in enumerate(subs) if s[0] != 'Z']
                row.append((subs, nz[0] * 128, (nz[-1] + 1) * 128))
        plan.append(row)
    return plan, tiles_list


def _build_nc(plan, nmt, has_vbias, exp_group=2, cast_engine="act",
              xbufs=10, ptbufs=4):
    nc = bacc.Bacc("TRN2", target_bir_lowering=False, debug=False)

    xqT = nc.dram_tensor("xqT", [D, S], F32R, kind="ExternalInput").ap()
    xkT = nc.dram_tensor("xkT", [D, S], F32R, kind="ExternalInput").ap()
    wq_d = nc.dram_tensor("wq", [D, HF], F32R, kind="ExternalInput").ap()
    wk_d = nc.dram_tensor("wk", [D, HF], F32R, kind="ExternalInput").ap()
    wkv_d = nc.dram_tensor("wkv", [D, HF], F32R, kind="ExternalInput").ap()
    wo_d = nc.dram_tensor("wo", [HF, D], BF16, kind="ExternalInput").ap()
    bq_d = nc.dram_tensor("bq", [128, 2], F32, kind="ExternalInput").ap()
    bk_d = nc.dram_tensor("bk", [128, 2], F32, kind="ExternalInput").ap()
    bkv_d = nc.dram_tensor("bkv", [1, HF], F32R, kind="ExternalInput").ap()
    mt_d = nc.dram_tensor("mtiles", [max(nmt, 1), 128, 128], BF16,
                          kind="ExternalInput").ap()
    out_d = nc.dram_tensor("out", [S, D], F32, kind="ExternalOutput").ap()

    with tile.TileContext(nc) as tc:
        with tc.tile_pool(name="wpool", bufs=1) as wpool, \
             tc.tile_pool(name="big", bufs=1) as big, \
             tc.tile_pool(name="xpool", bufs=xbufs) as xpool, \
             tc.tile_pool(name="ptpool", bufs=ptbufs) as ptpool, \
             tc.tile_pool(name="npool", bufs=2) as npool, \
             tc.tile_pool(name="opool", bufs=3) as opool, \
             tc.tile_pool(name="pspool", bufs=4, space="PSUM") as pspool, \
             tc.tile_pool(name="pspool2", bufs=2, space="PSUM") as pspool2:

            def cast_bias(dst, src, bias_ap):
                if cast_engine == "act":
                    nc.scalar.activation(dst, src, AF.Identity,
                                         bias=bias_ap, scale=1.0)
                else:
                    nc.vector.tensor_scalar_add(dst, src, bias_ap)

            # ---------------- weights / constants ----------------
            # Loaded just-in-time (emission order = DMA queue order): wq/bq
            # before q-proj, wk/wkv before kv-proj, wo/masks before first use.
            wq_sb = wpool.tile([128, NDC * HF], F32R, tag="wq")
            wk_sb = wpool.tile([128, NDC * HF], F32R, tag="wk")
            wkv_sb = wpool.tile([128, NDC * HF], F32R, tag="wkv")
            wo_sb = wpool.tile([128, 2 * D], BF16, tag="wo")
            bq_sb = wpool.tile([128, 2], F32, tag="bq")
            bk_sb = wpool.tile([128, 2], F32, tag="bk")
            mt_sb = None
            if nmt > 0:
                mt_sb = wpool.tile([128, nmt * 128], BF16, tag="mt",
                                   name="mt_sb")
            if has_vbias:
                ones_sb = wpool.tile([1, 128], F32R, tag="ones")
                bkv_sb = wpool.tile([1, HF], F32R, tag="bkv")

            def emit_load_q_weights():
                nc.sync.dma_start(bq_sb[:], bq_d)
                for kc in range(NDC):
                    nc.sync.dma_start(wq_sb[:, kc * HF:(kc + 1) * HF],
                                      wq_d[kc * 128:(kc + 1) * 128, :])

            def emit_load_kv_weights(kc):
                if kc == 0:
                    nc.sync.dma_start(bk_sb[:], bk_d)
                nc.sync.dma_start(wk_sb[:, kc * HF:(kc + 1) * HF],
                                  wk_d[kc * 128:(kc + 1) * 128, :])
                nc.sync.dma_start(wkv_sb[:, kc * HF:(kc + 1) * HF],
                                  wkv_d[kc * 128:(kc + 1) * 128, :])
                if has_vbias and kc == 0:
                    nc.gpsimd.memset(ones_sb[:], 1.0)
                    nc.sync.dma_start(bkv_sb[:], bkv_d)

            def emit_load_masks():
                for i in range(nmt):
                    nc.sync.dma_start(mt_sb[:, i * 128:(i + 1) * 128], mt_d[i])

            def emit_load_wo():
                for hb in range(2):
                    nc.sync.dma_start(wo_sb[:, hb * D:(hb + 1) * D],
                                      wo_d[hb * 128:(hb + 1) * 128, :])

            # ---------------- persistent activations ----------------
            qT_sb = big.tile([128, 2 * S], BF16, tag="qT")
            kT_sb = big.tile([128, 2 * S], BF16, tag="kT")
            vaug_sb = big.tile([128, NKC * 512], BF16, tag="vaug")
            attnT_sb = big.tile([128, 2 * S], BF16, tag="attnT")

            # ones blocks of v_aug: per kchunk, per head: cols [.. +64 .. +128)
            for kc in range(NKC):
                for h in range(HPC):
                    nc.gpsimd.memset(
                        vaug_sb[:, kc * 512 + h * 128 + 64: kc * 512 + h * 128 + 128],
                        1.0)

            # ---------------- projection emitters ----------------
            # q projection for one 512-token chunk: two per-hb psum singles.
            def emit_qproj(tci, weave_dma=None):
                psq = [pspool.tile([128, 512], F32, tag="ps",
                                   name=f"psq{tci}_{hb}") for hb in range(2)]
                for kc in range(NDC):
                    xq_t = xpool.tile([128, 512], F32R, tag="x",
                                      name=f"xq{tci}_{kc}")
                    nc.sync.dma_start(
                        xq_t[:], xqT[kc * 128:(kc + 1) * 128,
                                     tci * 512:(tci + 1) * 512])
                    if weave_dma is not None:
                        weave_dma(kc)
                    for hb in range(2):
                        nc.tensor.matmul(
                            psq[hb][:],
                            wq_sb[:, kc * HF + hb * 128: kc * HF + (hb + 1) * 128],
                            xq_t[:],
                            start=(kc == 0), stop=(kc == NDC - 1))
                for hb in range(2):
                    cast_bias(
                        qT_sb[:, hb * S + tci * 512: hb * S + (tci + 1) * 512],
                        psq[hb][:], bq_sb[:, hb:hb + 1])

            # k+v projections for one 512-token chunk; xk tiles stay live
            # across the kc loop so v accumulates one psv at a time.
            def emit_kvproj(tci):
                psk = [pspool.tile([128, 512], F32, tag="ps",
                                   name=f"psk{tci}_{hb}") for hb in range(2)]
                xks = []
                for kc in range(NDC):
                    xk_t = xpool.tile([128, 512], F32R, tag="x",
                                      name=f"xk{tci}_{kc}")
                    nc.sync.dma_start(
                        xk_t[:], xkT[kc * 128:(kc + 1) * 128,
                                     tci * 512:(tci + 1) * 512])
                    xks.append(xk_t)
                    for hb in range(2):
                        nc.tensor.matmul(
                            psk[hb][:],
                            wk_sb[:, kc * HF + hb * 128: kc * HF + (hb + 1) * 128],
                            xk_t[:],
                            start=(kc == 0), stop=(kc == NDC - 1))
                for hb in range(2):
                    cast_bias(
                        kT_sb[:, hb * S + tci * 512: hb * S + (tci + 1) * 512],
                        psk[hb][:], bk_sb[:, hb:hb + 1])
                for ts in range(4):
                    psv = pspool.tile([128, HF], F32, tag="ps",
                                      name=f"psv{tci}_{ts}")
                    for kc in range(NDC):
                        nc.tensor.matmul(
                            psv[:],
                            xks[kc][:, ts * 128:(ts + 1) * 128],
                            wkv_sb[:, kc * HF:(kc + 1) * HF],
                            start=(kc == 0),
                            stop=(kc == NDC - 1 and not has_vbias))
                    if has_vbias:
                        nc.tensor.matmul(psv[:], ones_sb[:], bkv_sb[:],
                                         start=False, stop=True)
                    kci = tci * 4 + ts
                    for h in range(HPC):
                        nc.vector.tensor_copy(
                            vaug_sb[:, kci * 512 + h * 128: kci * 512 + h * 128 + 64],
                            psv[:, h * 64:(h + 1) * 64])

            # output projection for one 128-token chunk of a finished q-block
            def emit_oproj(tok0):
                for of in range(2):
                    ops = pspool.tile([128, 512], F32, tag="ps")
                    for hb2 in range(2):
                        nc.tensor.matmul(
                            ops[:],
                            attnT_sb[:, hb2 * S + tok0: hb2 * S + tok0 + 128],
                            wo_sb[:, hb2 * D + of * 512: hb2 * D + (of + 1) * 512],
                            start=(hb2 == 0), stop=(hb2 == 1))
                    obuf = opool.tile([128, 512], F32, tag="obuf")
                    nc.vector.tensor_copy(obuf[:], ops[:])
                    nc.sync.dma_start(
                        out_d[tok0:tok0 + 128, of * 512:(of + 1) * 512],
                        obuf[:])

            # ---------------- attention ----------------
            # Pair k-chunks so one wide ACT exp covers two 512-col S^T tiles
            # (amortizes the ~352-cycle ACT instruction setup).
            def emit_attention(qb, extras):
                q0 = qb * 512
                kcs = [kc for kc in range(NKC) if plan[qb][kc] is not None]
                groups = [kcs[i:i + exp_group]
                          for i in range(0, len(kcs), exp_group)]
                for h in range(HPC):
                    hb, hr = h // 2, (h % 2) * 64
                    pv_ps = pspool.tile([128, 512], F32, tag="ps",
                                        name=f"pv{qb}_{h}")
                    for grp in groups:
                        g = len(grp)
                        pool_g = pspool2 if g > 1 else pspool
                        st_ps = pool_g.tile([128, 512 * g], F32,
                                            tag="ps2" if g > 1 else "ps")
                        pt = ptpool.tile([128, 512 * g], BF16, tag="pt")
                        for i, kc in enumerate(grp):
                            nc.tensor.matmul(
                                st_ps[:, i * 512:(i + 1) * 512],
                                kT_sb[hr:hr + 64,
                                      hb * S + kc * 128: hb * S + (kc + 1) * 128],
                                qT_sb[hr:hr + 64, hb * S + q0: hb * S + q0 + 512],
                                start=True, stop=True)
                        e0 = plan[qb][grp[0]][1]
                        e1 = (g - 1) * 512 + plan[qb][grp[-1]][2]
                        nc.scalar.activation(pt[:, e0:e1], st_ps[:, e0:e1],
                                             AF.Exp, bias=0.0,
                                             scale=float(SCALE))
                        for i, kc in enumerate(grp):
                            subs, c0, c1 = plan[qb][kc]
                            base = i * 512
                            # PV streams only [c0, c1); cols outside stay
                            # unread (masked-out contributions are skipped,
                            # not zeroed).  Interior all-zero subs still need
                            # a memset.  The overall-first k-chunk must cover
                            # the full 512 so every psum column is started.
                            if kc == kcs[0] and (c0 != 0 or c1 != 512):
                                if c0 > 0:
                                    nc.vector.memset(pt[:, base:base + c0], 0.0)
                                if c1 < 512:
                                    nc.vector.memset(pt[:, base + c1:base + 512], 0.0)
                                c0, c1 = 0, 512
                            for j, sub in enumerate(subs):
                                lo, hi = base + j * 128, base + (j + 1) * 128
                                if sub[0] == 'Z' and j * 128 >= c0 and (j + 1) * 128 <= c1:
                                    nc.vector.memset(pt[:, lo:hi], 0.0)
                                elif sub[0] == 'M':
                                    idx = sub[1]
                                    nc.vector.tensor_mul(
                                        pt[:, lo:hi], pt[:, lo:hi],
                                        mt_sb[:, idx * 128:(idx + 1) * 128])
                            nc.tensor.matmul(
                                pv_ps[:, c0:c1],
                                vaug_sb[:, kc * 512 + h * 128: kc * 512 + (h + 1) * 128],
                                pt[:, base + c0:base + c1],
                                start=(kc == kcs[0]), stop=(kc == kcs[-1]))
                    recip = npool.tile([64, 512], F32, tag="recip")
                    nc.vector.reciprocal(recip[:], pv_ps[64:128, :])
                    nc.vector.tensor_mul(
                        attnT_sb[hr:hr + 64, hb * S + q0: hb * S + q0 + 512],
                        pv_ps[0:64, :], recip[:])
                    if extras:
                        extras.pop(0)()
                while extras:
                    extras.pop(0)()

            # ---------------- emission order (weaving) ----------------
            # Early projections, then per q-block attention with later
            # projections / previous q-block output projections woven between
            # heads so PE has work while ACT chews through the exps.
            emit_load_q_weights()
            emit_qproj(0, weave_dma=emit_load_kv_weights)
            emit_kvproj(0)
            emit_load_masks()
            emit_attention(0, [lambda: emit_load_wo(),
                               lambda: emit_qproj(1), lambda: emit_kvproj(1),
                               lambda: emit_qproj(2), lambda: emit_kvproj(2)])
            emit_attention(1, [lambda: emit_qproj(3), lambda: emit_kvproj(3),
                               lambda: emit_oproj(0 * 512 + 0 * 128),
                               lambda: emit_oproj(0 * 512 + 1 * 128)])
            emit_attention(2, [lambda: emit_oproj(0 * 512 + 2 * 128),
                               lambda: emit_oproj(0 * 512 + 3 * 128),
                               lambda: emit_oproj(1 * 512 + 0 * 128),
                               lambda: emit_oproj(1 * 512 + 1 * 128)])
            emit_attention(3, [lambda: emit_oproj(1 * 512 + 2 * 128),
                               lambda: emit_oproj(1 * 512 + 3 * 128),
                               lambda: emit_oproj(2 * 512 + 0 * 128),
                               lambda: emit_oproj(2 * 512 + 1 * 128),
                               lambda: emit_oproj(2 * 512 + 2 * 128),
                               lambda: emit_oproj(2 * 512 + 3 * 128)])
            for t in range(4):
                emit_oproj(3 * 512 + t * 128)
    nc.compile()
    return nc


_CACHE = {}

# tunables (kept as module globals so experiments can override)
VARIANT = {"exp_group": 2, "cast_engine": "act", "xbufs": 10, "ptbufs": 4}


def _get_nc(plan, nmt, has_vbias):
    key = (repr(plan), nmt, has_vbias, repr(sorted(VARIANT.items())))
    if key not in _CACHE:
        _CACHE[key] = _build_nc(plan, nmt, has_vbias, **VARIANT)
    return _CACHE[key]


def shard_inputs(queries, keys, mask, Wq, bq, Wk, bk, Wv, bv, Wo, bo):
    """Host-side prep: returns (in_maps, plan, nmt, has_vbias)."""
    Wkv = (Wk.astype(np.float64) @ Wv.astype(np.float64)).astype(np.float32)
    bkv = (bk.astype(np.float64) @ Wv.astype(np.float64)
           + bv.astype(np.float64)).astype(np.float32)
    has_vbias = bool(np.any(bkv != 0.0))

    plan, tiles_list = _classify_mask(np.asarray(mask))
    nmt = len(tiles_list)
    assert nmt <= 64, f"too many distinct mask tiles ({nmt})"
    if nmt > 0:
        mtiles = np.stack(tiles_list).astype(ml_dtypes.bfloat16)
    else:
        mtiles = np.zeros((1, 128, 128), dtype=ml_dtypes.bfloat16)

    in_maps = []
    for c in range(NCORES):
        b, g = c // 4, c % 4
        cols = slice(HF * g, HF * (g + 1))
        in_maps.append({
            "xqT": np.ascontiguousarray(queries[b].T),
            "xkT": np.ascontiguousarray(keys[b].T),
            "wq": np.ascontiguousarray(Wq[:, cols]),
            "wk": np.ascontiguousarray(Wk[:, cols]),
            "wkv": np.ascontiguousarray(Wkv[:, cols]),
            "wo": np.ascontiguousarray(Wo[cols, :]).astype(ml_dtypes.bfloat16),
            "bq": np.ascontiguousarray(bq[cols].reshape(2, 128).T),
            "bk": np.ascontiguousarray(bk[cols].reshape(2, 128).T),
            "bkv": bkv[cols].reshape(1, HF).copy(),
            "mtiles": mtiles,
        })
    return in_maps, plan, nmt, has_vbias


def combine_outputs(results, bo):
    out = np.empty((B, S, D), dtype=np.float32)
    for b in range(B):
        acc = results[4 * b]["out"].astype(np.float32).copy()
        for g in range(1, 4):
            acc += results[4 * b + g]["out"]
        out[b] = acc + bo[None, :]
    return out


def kernel(queries, keys, values, mask, Wq, bq, Wk, bk, Wv, bv, Wo, bo,
           _trace=False, _result_holder=None):
    queries = np.asarray(queries, dtype=np.float32)
    keys = np.asarray(keys, dtype=np.float32)
    mask = np.asarray(mask)
    in_maps, plan, nmt, has_vbias = shard_inputs(
        queries, keys, mask,
        np.asarray(Wq, np.float32), np.asarray(bq, np.float32),
        np.asarray(Wk, np.float32), np.asarray(bk, np.float32),
        np.asarray(Wv, np.float32), np.asarray(bv, np.float32),
        np.asarray(Wo, np.float32), np.asarray(bo, np.float32))
    nc = _get_nc(plan, nmt, has_vbias)
    res = run_bass_kernel_spmd(nc, in_maps, core_ids=list(range(NCORES)),
                               trace=_trace)
    if _result_holder is not None:
        _result_holder.append(res)
    return combine_outputs(res.results, np.asarray(bo, np.float32))
